# revision 45
# baseline (speedup 1.0000x reference)
"""Masked multi-head SDP attention, 8 NeuronCores = (batch, head-half).

B=4, S=2048, D=1024, H=16, DK=64. Core c owns batch c//2 and heads
[(c%2)*8, (c%2)*8+8), processed as 4 head-pair groups. All matmuls bf16
(full PE rate at any free width; rel-err gate 2e-2 >> bf16 noise).

- x is cast to bf16 on host and loaded straight into [d, s] layout with
  the XBAR DMA-transpose, once per core, reused by all 4 groups (no PE
  transposes or psum->sbuf copies for x^T).
- Scores for both heads of a t-tile go into one [128, 1024] PSUM pair and
  through a single 2-region exp on the Act engine; causal masking is a
  Pool-engine multiply with a duplicated triu tile (Act is the binding
  engine in attention, PE overall).
- V is stored per t-tile as [v0 | ones | v1] so each head's attn@V lhsT
  is a contiguous 128-col window whose PSUM rows hold both v@attn and the
  softmax denominator replicated 64x; normalization is reciprocal + mul
  with cross-partition-base APs (no PE broadcast, no DMA hop).
- Emission is software-pipelined: all projection groups flow through one
  chained generator that attention pumps per i-block (so PE never waits
  for late x^T quarters) and pulls as per-j filler; output-projection
  halves are pushed as fillers into the last group's stream. attn@V lags
  scores by 3-4 t-tiles so it never stalls on exp.
- The output projection accumulates all 4 groups in PSUM; each core
  writes one [S, D] bf16 partial and the host sums the two halves per
  batch in f32.
"""

import sys

sys.path.insert(0, "/opt/trn_rl_repo")

import collections
import numpy as np
import ml_dtypes

import concourse.bass as bass
import concourse.mybir as mybir
from concourse import bacc
from concourse.masks import make_identity
from concourse.tile import TileContext
from concourse.bass_utils import run_bass_kernel_spmd

B, S, D, H = 4, 2048, 1024, 16
DK = D // H  # 64
NCORES = 8
NG = 4  # head-pair groups per core
KH = 2 * DK  # 128 per group
KC = NG * KH  # 512 projection outputs per core
NT = S // 128
NI = S // 512
DC = D // 128

F32 = mybir.dt.float32
BF16 = mybir.dt.bfloat16

VW = 3 * DK  # 192: [v0 | ones | v1] per t-tile in vna


def build_nc():
    nc = bacc.Bacc("TRN2", target_bir_lowering=False, debug=False,
                   num_devices=NCORES)
    x = nc.dram_tensor("x", [S, D], BF16, kind="ExternalInput").ap()
    wq = nc.dram_tensor("wq", [DC, 128, KC], BF16, kind="ExternalInput").ap()
    wk = nc.dram_tensor("wk", [DC, 128, KC], BF16, kind="ExternalInput").ap()
    wv = nc.dram_tensor("wv", [DC, 128, KC], BF16, kind="ExternalInput").ap()
    bq = nc.dram_tensor("bq", [128, NG], F32, kind="ExternalInput").ap()
    bk = nc.dram_tensor("bk", [128, NG], F32, kind="ExternalInput").ap()
    bv = nc.dram_tensor("bv", [128, NG], F32, kind="ExternalInput").ap()
    wo = nc.dram_tensor("wo", [NG, KH, D], BF16, kind="ExternalInput").ap()
    tri = nc.dram_tensor("tri", [128, 128], BF16, kind="ExternalInput").ap()
    out = nc.dram_tensor("out", [S, D], BF16, kind="ExternalOutput").ap()

    with TileContext(nc) as tc:
        with (
            tc.tile_pool(name="const", bufs=1) as cpool,
            tc.tile_pool(name="seq", bufs=2) as qpool,
            tc.tile_pool(name="vn", bufs=2) as vpool,
            tc.tile_pool(name="attn", bufs=1) as apool,
            tc.tile_pool(name="fin", bufs=2) as fpool,
            tc.tile_pool(name="pacc", bufs=2, space="PSUM") as ps_acc,
            tc.tile_pool(name="psc", bufs=2, space="PSUM") as ps_sc,
            tc.tile_pool(name="pv", bufs=1, space="PSUM") as ps_v,
        ):
            # x^T quarter 0 first, then weights, then remaining quarters —
            # minimizes time until the first projection group can run
            # (HWDGE issue is serialized at ~625ns/DMA).
            xts = [cpool.tile([128, S], BF16, tag=f"xt{dc}",
                              name=f"xt_{dc}") for dc in range(DC)]

            def xt_quarter(sh):
                for dc in range(DC):
                    nc.sync.dma_start(
                        out=xts[dc][:, sh * 512:(sh + 1) * 512],
                        in_=x[sh * 512:(sh + 1) * 512,
                              dc * 128:(dc + 1) * 128],
                        transpose=True)

            w_sb = {}
            for nm, src in (("q", wq), ("k", wk), ("v", wv)):
                t = cpool.tile([128, DC * KC], BF16, tag="w" + nm,
                               name=f"w_{nm}")
                nc.sync.dma_start(
                    out=t[:].rearrange("p (c k) -> p c k", c=DC),
                    in_=src.rearrange("c p k -> p c k"))
                w_sb[nm] = t
            b_sb = {}
            for nm, src in (("q", bq), ("k", bk), ("v", bv)):
                t = cpool.tile([128, NG], F32, tag="b" + nm, name=f"b_{nm}")
                nc.sync.dma_start(out=t[:], in_=src)
                b_sb[nm] = t
            ident = cpool.tile([128, 128], BF16, tag="ident")
            make_identity(nc, ident[:])
            tri2_sb = cpool.tile([128, 256], BF16, tag="tri")
            nc.sync.dma_start(out=tri2_sb[:, 0:128], in_=tri)
            nc.sync.dma_start(out=tri2_sb[:, 128:256], in_=tri)
            for _sh in range(4):
                xt_quarter(_sh)
            wo_sb = [cpool.tile([KH, D], BF16, tag=f"wo{g}", name=f"wo_{g}")
                     for g in range(NG)]
            for g in range(NG):
                nc.sync.dma_start(out=wo_sb[g][:], in_=wo[g])

            def prepare(g):
                """Per-group tiles (vna ones, qt, kt)."""
                vna = vpool.tile([128, NT * VW], BF16, tag="vna",
                                 name=f"vna_{g}")
                vna_r = vna[:].rearrange("p (j g c) -> p j g c", j=NT, g=3)
                nc.gpsimd.memset(vna_r[:, :, 1:2, :], 1.0)
                qt = qpool.tile([128, S], BF16, tag="qt", name=f"qt_{g}")
                kt = qpool.tile([128, S], BF16, tag="kt", name=f"kt_{g}")
                return dict(g=g, vna=vna, vna_r=vna_r, qt=qt, kt=kt)

            def proj_gen(ctx):
                g = ctx["g"]
                for st in range(NI):
                    sl = slice(st * 512, (st + 1) * 512)
                    for nm in ("q", "k", "v"):
                        acc = ps_acc.tile([128, 512], F32, tag="acc",
                                          name=f"acc_{g}_{st}_{nm}")
                        for dc0 in range(0, DC, 2):
                            for dc in (dc0, dc0 + 1):
                                nc.tensor.matmul(
                                    acc[:],
                                    w_sb[nm][:, dc * KC + g * KH:
                                             dc * KC + (g + 1) * KH],
                                    xts[dc][:, sl], start=(dc == 0),
                                    stop=(dc == DC - 1))
                            yield
                        if nm != "v":
                            dst = ctx["qt"] if nm == "q" else ctx["kt"]
                            nc.vector.tensor_scalar_add(
                                dst[:, sl], acc[:], b_sb[nm][:, g:g + 1])
                        else:
                            vtt = fpool.tile([128, 512], BF16, tag="vtt",
                                             name=f"vtt_{g}_{st}")
                            nc.vector.tensor_scalar_add(
                                vtt[:], acc[:], b_sb["v"][:, g:g + 1])
                            tp = ps_acc.tile([128, 512], F32, tag="acc",
                                             name=f"tp_{g}_{st}")
                            tpb = tp[:].bitcast(BF16)
                            for q in range(4):
                                nc.tensor.transpose(
                                    tpb[:, q * 128:(q + 1) * 128],
                                    vtt[:, q * 128:(q + 1) * 128], ident[:])
                            yield
                            tp_r = tpb[:, 0:512].rearrange(
                                "p (q g c) -> p q g c", q=4, g=2)
                            js = slice(st * 4, (st + 1) * 4)
                            nc.vector.tensor_copy(
                                ctx["vna_r"][:, js, 0:1, :],
                                tp_r[:, :, 0:1, :])
                            nc.vector.tensor_copy(
                                ctx["vna_r"][:, js, 2:3, :],
                                tp_r[:, :, 1:2, :])
                            yield

            oneshot = collections.deque()
            gen_box = [None]

            def pull_gen(n=1):
                for _ in range(n):
                    if gen_box[0] is not None:
                        try:
                            next(gen_box[0])
                            continue
                        except StopIteration:
                            gen_box[0] = None
                    if oneshot:
                        oneshot.popleft()()

            def drain_all():
                while oneshot or gen_box[0] is not None:
                    pull_gen(1)

            def flush_v(vps, vna, item, nj):
                j, off, at = item
                for h in range(2):
                    base = j * VW + h * DK
                    nc.tensor.matmul(
                        vps[h][:, off:512], vna[:, base:base + 2 * DK],
                        at[:, h * 512 + off:h * 512 + 512],
                        start=(j == 0), stop=(j == nj - 1))

            catts = [None] * NG

            def outproj_half(st, half, tail=False):
                """pw[128,512] = sum_g catt_g[:, st] @ wo_g[:, half]."""
                def emit():
                    ob = obs[st]
                    pw = ps_acc.tile([128, 512], F32, tag="acc",
                                     name=f"pw_{st}_{half}")
                    for g in range(NG):
                        nc.tensor.matmul(
                            pw[:], catts[g][:, st * 128:(st + 1) * 128],
                            wo_sb[g][:, half * 512:(half + 1) * 512],
                            start=(g == 0), stop=(g == NG - 1))
                    dst = ob[:, half * 512:(half + 1) * 512]
                    if tail and half == 0:
                        nc.scalar.copy(dst, pw[:])
                    else:
                        nc.vector.tensor_copy(dst, pw[:])
                    if half == 1:
                        nc.sync.dma_start(
                            out=out[st * 128:(st + 1) * 128, :], in_=ob[:])
                return emit

            obs = {}

            def attention(ctx, pump=None):
                g = ctx["g"]
                qt, kt, vna = ctx["qt"], ctx["kt"], ctx["vna"]
                catt = fpool.tile([128, S], BF16, tag=f"catt{g}",
                                  name=f"catt_{g}")
                catts[g] = catt
                last = g == NG - 1
                for i in range(NI):
                    if pump is not None:
                        pump(i)
                    nj = 4 * i + 4
                    vps = [ps_v.tile([128, 512], F32, tag=f"v{h}",
                                     name=f"vp_{g}_{i}_{h}")
                           for h in range(2)]
                    sq0 = i * 512
                    pend = []
                    for j in range(nj):
                        q = j - 4 * i
                        off = 128 * q if q >= 0 else 0
                        sp = ps_sc.tile([128, 1024], F32, tag="sc",
                                        name=f"sp_{g}_{i}_{j}")
                        for h in range(2):
                            ks = slice(h * DK, (h + 1) * DK)
                            nc.tensor.matmul(
                                sp[:, h * 512 + off:h * 512 + 512],
                                kt[ks, j * 128:(j + 1) * 128],
                                qt[ks, sq0 + off:sq0 + 512],
                                start=True, stop=True)
                        at = apool.tile([128, 1024], BF16, tag=f"at{j}",
                                        name=f"at_{g}_{i}_{j}")
                        sp2 = sp[:].rearrange("p (r c) -> p r c", r=2)
                        at2 = at[:].rearrange("p (r c) -> p r c", r=2)
                        nc.scalar.activation(
                            at2[:, :, off:512], sp2[:, :, off:512],
                            mybir.ActivationFunctionType.Exp, scale=0.125)
                        if q >= 0:
                            nc.gpsimd.tensor_mul(
                                at2[:, :, off:off + 128],
                                at2[:, :, off:off + 128],
                                tri2_sb[:].rearrange("p (r c) -> p r c", r=2))
                        pend.append((j, off, at))
                        if len(pend) > (4 if i <= 1 else 3):
                            flush_v(vps, vna, pend.pop(0), nj)
                        pull_gen(1)
                    while pend:
                        flush_v(vps, vna, pend.pop(0), nj)

                    rcp = fpool.tile([128, 512], F32, tag="rcp",
                                     name=f"rcp_{g}_{i}")
                    nc.vector.reciprocal(rcp[0:64, :], vps[0][64:128, :])
                    nc.vector.reciprocal(rcp[64:128, :], vps[1][0:64, :])
                    nc.vector.tensor_mul(catt[0:64, sq0:sq0 + 512],
                                         vps[0][0:64, :], rcp[0:64, :])
                    nc.vector.tensor_mul(catt[64:128, sq0:sq0 + 512],
                                         vps[1][64:128, :], rcp[64:128, :])
                    if last:
                        tail = i == NI - 1
                        for st in range(4 * i, 4 * i + 4):
                            obs[st] = fpool.tile([128, D], BF16, tag="ob",
                                                 name=f"ob_{st}")
                            for half in range(2):
                                oneshot.append(
                                    outproj_half(st, half, tail=tail))
                    pull_gen(4)

            # All projection groups flow through one chained generator.
            # attention(g) pumps it just far enough that i-block k's inputs
            # (st-groups <= k of group g) are emitted, then keeps pulling it
            # as per-j filler — so PE never waits for late x^T quarters and
            # group g+1's projections interleave into group g's attention.
            YPG = 14  # proj_gen yields per st-group
            progress = [0] * NG
            ctxs = {}

            def chain():
                for gg in range(NG):
                    ctxs[gg] = prepare(gg)
                    for item in proj_gen(ctxs[gg]):
                        progress[gg] += 1
                        yield item

            gen_box[0] = chain()

            def pump(g, i):
                target = (i + 1) * YPG
                while gen_box[0] is not None and progress[g] < target:
                    pull_gen(1)

            for g in range(NG):
                while g not in ctxs and gen_box[0] is not None:
                    pull_gen(1)
                attention(ctxs[g], pump=lambda i, g=g: pump(g, i))
            drain_all()
    nc.finalize()
    return nc


_NC_CACHE = {}


def _get_nc():
    if "nc" not in _NC_CACHE:
        _NC_CACHE["nc"] = build_nc()
    return _NC_CACHE["nc"]


def kernel(x, Wq, bq, Wk, bk, Wv, bv, Wo, bo):
    x_bf = np.ascontiguousarray(np.asarray(x, dtype=np.float32)).astype(
        ml_dtypes.bfloat16)
    tri = np.triu(np.ones((128, 128), dtype=np.float32)).astype(
        ml_dtypes.bfloat16)
    in_maps = []
    for c in range(NCORES):
        b, half = c // 2, c % 2
        hs = [half * 8 + k for k in range(8)]
        m = {"x": x_bf[b], "tri": tri}
        # wo: per group g, rows for heads (2g, 2g+1) of this half
        wo_g = np.stack([
            np.concatenate([Wo[hs[2 * g] * DK:(hs[2 * g] + 1) * DK],
                            Wo[hs[2 * g + 1] * DK:(hs[2 * g + 1] + 1) * DK]],
                           axis=0)
            for g in range(NG)])
        m["wo"] = np.ascontiguousarray(wo_g.astype(ml_dtypes.bfloat16))
        for nm, W, bb in (("q", Wq, bq), ("k", Wk, bk), ("v", Wv, bv)):
            Wc = np.concatenate([W[h] for h in hs], axis=1)  # [D, 512]
            m["w" + nm] = np.ascontiguousarray(
                Wc.reshape(DC, 128, KC).astype(ml_dtypes.bfloat16))
            bc = np.concatenate([bb[h] for h in hs])  # [512]
            m["b" + nm] = np.ascontiguousarray(
                bc.reshape(NG, 128).T.astype(np.float32))
        in_maps.append(m)
    nc = _get_nc()
    res = run_bass_kernel_spmd(nc, in_maps, list(range(NCORES)))
    outp = np.zeros((B, S, D), dtype=np.float32)
    for c in range(NCORES):
        outp[c // 2] += np.asarray(res.results[c]["out"], dtype=np.float32)
    return outp + np.asarray(bo, dtype=np.float32)[None, None, :]


# revision 48
# speedup vs baseline: 1.0438x; 1.0438x over previous
"""Masked multi-head SDP attention, 8 NeuronCores = (batch, head-half).

B=4, S=2048, D=1024, H=16, DK=64. Core c owns batch c//2 and heads
[(c%2)*8, (c%2)*8+8), processed as 4 head-pair groups. All matmuls bf16
(full PE rate at any free width; rel-err gate 2e-2 >> bf16 noise).

- x is cast to bf16 on host and loaded straight into [d, s] layout with
  the XBAR DMA-transpose, once per core, reused by all 4 groups (no PE
  transposes or psum->sbuf copies for x^T).
- Scores for both heads of a t-tile go into one [128, 1024] PSUM pair and
  through a single 2-region exp on the Act engine; causal masking is a
  Pool-engine multiply with a duplicated triu tile (Act is the binding
  engine in attention, PE overall).
- V is stored per t-tile as [v0 | ones | v1] so each head's attn@V lhsT
  is a contiguous 128-col window whose PSUM rows hold both v@attn and the
  softmax denominator replicated 64x; normalization is reciprocal + mul
  with cross-partition-base APs (no PE broadcast, no DMA hop).
- Emission is software-pipelined: all projection groups flow through one
  chained generator that attention pumps per i-block (so PE never waits
  for late x^T quarters) and pulls as per-j filler; output-projection
  halves are pushed as fillers into the last group's stream. attn@V lags
  scores by 3-4 t-tiles so it never stalls on exp.
- The output projection accumulates all 4 groups in PSUM; each core
  writes one [S, D] bf16 partial and the host sums the two halves per
  batch in f32.
"""

import sys

sys.path.insert(0, "/opt/trn_rl_repo")

import collections
import numpy as np
import ml_dtypes

import concourse.bass as bass
import concourse.mybir as mybir
from concourse import bacc
from concourse.masks import make_identity
from concourse.tile import TileContext
from concourse.bass_utils import run_bass_kernel_spmd

B, S, D, H = 4, 2048, 1024, 16
DK = D // H  # 64
NCORES = 8
NG = 4  # head-pair groups per core
KH = 2 * DK  # 128 per group
KC = NG * KH  # 512 projection outputs per core
NT = S // 128
NI = S // 512
DC = D // 128

F32 = mybir.dt.float32
BF16 = mybir.dt.bfloat16

VW = 3 * DK  # 192: [v0 | ones | v1] per t-tile in vna


def build_nc():
    nc = bacc.Bacc("TRN2", target_bir_lowering=False, debug=False,
                   num_devices=NCORES)
    x = nc.dram_tensor("x", [S, D], BF16, kind="ExternalInput").ap()
    wq = nc.dram_tensor("wq", [DC, 128, KC], BF16, kind="ExternalInput").ap()
    wk = nc.dram_tensor("wk", [DC, 128, KC], BF16, kind="ExternalInput").ap()
    wv = nc.dram_tensor("wv", [DC, 128, KC], BF16, kind="ExternalInput").ap()
    bq = nc.dram_tensor("bq", [128, NG], F32, kind="ExternalInput").ap()
    bk = nc.dram_tensor("bk", [128, NG], F32, kind="ExternalInput").ap()
    bv = nc.dram_tensor("bv", [128, NG], F32, kind="ExternalInput").ap()
    wo = nc.dram_tensor("wo", [NG, KH, D], BF16, kind="ExternalInput").ap()
    tri = nc.dram_tensor("tri", [128, 128], BF16, kind="ExternalInput").ap()
    out = nc.dram_tensor("out", [S, D], BF16, kind="ExternalOutput").ap()

    with TileContext(nc) as tc:
        with (
            tc.tile_pool(name="const", bufs=1) as cpool,
            tc.tile_pool(name="seq", bufs=2) as qpool,
            tc.tile_pool(name="vn", bufs=2) as vpool,
            tc.tile_pool(name="attn", bufs=2) as apool,
            tc.tile_pool(name="fin", bufs=4) as fpool,
            tc.tile_pool(name="cat", bufs=1) as catpool,
            tc.tile_pool(name="pacc", bufs=2, space="PSUM") as ps_acc,
            tc.tile_pool(name="psc", bufs=2, space="PSUM") as ps_sc,
            tc.tile_pool(name="pv", bufs=1, space="PSUM") as ps_v,
        ):
            # x^T quarter 0 first, then weights, then remaining quarters —
            # minimizes time until the first projection group can run
            # (HWDGE issue is serialized at ~625ns/DMA).
            xts = [cpool.tile([128, S], BF16, tag=f"xt{dc}",
                              name=f"xt_{dc}") for dc in range(DC)]

            def xt_quarter(sh):
                for dc in range(DC):
                    nc.sync.dma_start(
                        out=xts[dc][:, sh * 512:(sh + 1) * 512],
                        in_=x[sh * 512:(sh + 1) * 512,
                              dc * 128:(dc + 1) * 128],
                        transpose=True)

            w_sb = {}
            for nm, src in (("q", wq), ("k", wk), ("v", wv)):
                t = cpool.tile([128, DC * KC], BF16, tag="w" + nm,
                               name=f"w_{nm}")
                nc.sync.dma_start(
                    out=t[:].rearrange("p (c k) -> p c k", c=DC),
                    in_=src.rearrange("c p k -> p c k"))
                w_sb[nm] = t
                if nm == "q":
                    # x rows 0:512 in natural layout (cheap contiguous DMA);
                    # the otherwise-idle PE transposes them into x^T quarter 0
                    # while wk/wv stream in
                    xn0 = cpool.tile([128, 4 * D], BF16, tag="xn0")
                    nc.sync.dma_start(
                        out=xn0[:].rearrange("p (s d) -> p s d", s=4),
                        in_=x[0:512, :].rearrange("(s p) d -> p s d", p=128))
            b_sb = {}
            for nm, src in (("q", bq), ("k", bk), ("v", bv)):
                t = cpool.tile([128, NG], F32, tag="b" + nm, name=f"b_{nm}")
                nc.sync.dma_start(out=t[:], in_=src)
                b_sb[nm] = t
            ident = cpool.tile([128, 128], BF16, tag="ident")
            make_identity(nc, ident[:])
            tri2_sb = cpool.tile([128, 256], BF16, tag="tri")
            nc.sync.dma_start(out=tri2_sb[:, 0:128], in_=tri)
            nc.sync.dma_start(out=tri2_sb[:, 128:256], in_=tri)
            # PE-transpose x^T quarter 0 from the natural-layout load (the
            # PE is otherwise idle while wk/wv stream in); quarters 1-3 use
            # the XBAR DMA-transpose as before.
            for dc in range(DC):
                tp0 = ps_acc.tile([128, 512], F32, tag="acc",
                                  name=f"tp0_{dc}")
                tp0b = tp0[:].bitcast(BF16)
                for ss in range(4):
                    nc.tensor.transpose(
                        tp0b[:, ss * 128:(ss + 1) * 128],
                        xn0[:, ss * D + dc * 128:ss * D + (dc + 1) * 128],
                        ident[:])
                nc.vector.tensor_copy(xts[dc][:, 0:512], tp0b[:, 0:512])
            for _sh in range(1, 4):
                xt_quarter(_sh)
            wo_sb = [cpool.tile([KH, D], BF16, tag=f"wo{g}", name=f"wo_{g}")
                     for g in range(NG)]
            for g in range(NG):
                nc.sync.dma_start(out=wo_sb[g][:], in_=wo[g])

            def prepare(g):
                """Per-group tiles (vna ones, qt, kt)."""
                vna = vpool.tile([128, NT * VW], BF16, tag="vna",
                                 name=f"vna_{g}")
                vna_r = vna[:].rearrange("p (j g c) -> p j g c", j=NT, g=3)
                nc.gpsimd.memset(vna_r[:, :, 1:2, :], 1.0)
                qt = qpool.tile([128, S], BF16, tag="qt", name=f"qt_{g}")
                kt = qpool.tile([128, S], BF16, tag="kt", name=f"kt_{g}")
                return dict(g=g, vna=vna, vna_r=vna_r, qt=qt, kt=kt)

            def proj_gen(ctx):
                g = ctx["g"]
                for st in range(NI):
                    sl = slice(st * 512, (st + 1) * 512)
                    for nm in ("q", "k", "v"):
                        acc = ps_acc.tile([128, 512], F32, tag="acc",
                                          name=f"acc_{g}_{st}_{nm}")
                        for dc0 in range(0, DC, 2):
                            for dc in (dc0, dc0 + 1):
                                nc.tensor.matmul(
                                    acc[:],
                                    w_sb[nm][:, dc * KC + g * KH:
                                             dc * KC + (g + 1) * KH],
                                    xts[dc][:, sl], start=(dc == 0),
                                    stop=(dc == DC - 1))
                            yield
                        if nm != "v":
                            dst = ctx["qt"] if nm == "q" else ctx["kt"]
                            nc.vector.tensor_scalar_add(
                                dst[:, sl], acc[:], b_sb[nm][:, g:g + 1])
                        else:
                            vtt = fpool.tile([128, 512], BF16, tag="vtt",
                                             name=f"vtt_{g}_{st}")
                            nc.vector.tensor_scalar_add(
                                vtt[:], acc[:], b_sb["v"][:, g:g + 1])
                            tp = ps_acc.tile([128, 512], F32, tag="acc",
                                             name=f"tp_{g}_{st}")
                            tpb = tp[:].bitcast(BF16)
                            for q in range(4):
                                nc.tensor.transpose(
                                    tpb[:, q * 128:(q + 1) * 128],
                                    vtt[:, q * 128:(q + 1) * 128], ident[:])
                            yield
                            tp_r = tpb[:, 0:512].rearrange(
                                "p (q g c) -> p q g c", q=4, g=2)
                            js = slice(st * 4, (st + 1) * 4)
                            nc.vector.tensor_copy(
                                ctx["vna_r"][:, js, 0:1, :],
                                tp_r[:, :, 0:1, :])
                            nc.vector.tensor_copy(
                                ctx["vna_r"][:, js, 2:3, :],
                                tp_r[:, :, 1:2, :])
                            yield

            oneshot = collections.deque()
            gen_box = [None]

            def pull_gen(n=1):
                for _ in range(n):
                    if gen_box[0] is not None:
                        try:
                            next(gen_box[0])
                            continue
                        except StopIteration:
                            gen_box[0] = None
                    if oneshot:
                        oneshot.popleft()()

            def drain_all():
                while oneshot or gen_box[0] is not None:
                    pull_gen(1)

            def flush_v(vps, vna, item, nj):
                j, off, at = item
                for h in range(2):
                    base = j * VW + h * DK
                    nc.tensor.matmul(
                        vps[h][:, off:512], vna[:, base:base + 2 * DK],
                        at[:, h * 512 + off:h * 512 + 512],
                        start=(j == 0), stop=(j == nj - 1))

            catts = [None] * NG

            def outproj_half(st, half, tail=False):
                """pw[128,512] = sum_g catt_g[:, st] @ wo_g[:, half]."""
                def emit():
                    ob = obs[st]
                    pw = ps_acc.tile([128, 512], F32, tag="acc",
                                     name=f"pw_{st}_{half}")
                    for g in range(NG):
                        nc.tensor.matmul(
                            pw[:], catts[g][:, st * 128:(st + 1) * 128],
                            wo_sb[g][:, half * 512:(half + 1) * 512],
                            start=(g == 0), stop=(g == NG - 1))
                    dst = ob[:, half * 512:(half + 1) * 512]
                    if tail and half == 0:
                        nc.scalar.copy(dst, pw[:])
                    else:
                        nc.vector.tensor_copy(dst, pw[:])
                    if half == 1:
                        nc.sync.dma_start(
                            out=out[st * 128:(st + 1) * 128, :], in_=ob[:])
                return emit

            obs = {}

            def attention(ctx, pump=None):
                g = ctx["g"]
                qt, kt, vna = ctx["qt"], ctx["kt"], ctx["vna"]
                catt = catpool.tile([128, S], BF16, tag=f"catt{g}",
                                    name=f"catt_{g}")
                catts[g] = catt
                last = g == NG - 1
                for i in range(NI):
                    if pump is not None:
                        pump(i)
                    nj = 4 * i + 4
                    vps = [ps_v.tile([128, 512], F32, tag=f"v{h}",
                                     name=f"vp_{g}_{i}_{h}")
                           for h in range(2)]
                    sq0 = i * 512
                    pend = []
                    for j in range(nj):
                        q = j - 4 * i
                        off = 128 * q if q >= 0 else 0
                        sp = ps_sc.tile([128, 1024], F32, tag="sc",
                                        name=f"sp_{g}_{i}_{j}")
                        for h in range(2):
                            ks = slice(h * DK, (h + 1) * DK)
                            nc.tensor.matmul(
                                sp[:, h * 512 + off:h * 512 + 512],
                                kt[ks, j * 128:(j + 1) * 128],
                                qt[ks, sq0 + off:sq0 + 512],
                                start=True, stop=True)
                        at = apool.tile([128, 1024], BF16, tag=f"at{j}",
                                        name=f"at_{g}_{i}_{j}")
                        sp2 = sp[:].rearrange("p (r c) -> p r c", r=2)
                        at2 = at[:].rearrange("p (r c) -> p r c", r=2)
                        nc.scalar.activation(
                            at2[:, :, off:512], sp2[:, :, off:512],
                            mybir.ActivationFunctionType.Exp, scale=0.125)
                        if q >= 0:
                            nc.vector.tensor_mul(
                                at2[:, :, off:off + 128],
                                at2[:, :, off:off + 128],
                                tri2_sb[:].rearrange("p (r c) -> p r c", r=2))
                        pend.append((j, off, at))
                        if len(pend) > 3:
                            flush_v(vps, vna, pend.pop(0), nj)
                        pull_gen(1)
                    while pend:
                        flush_v(vps, vna, pend.pop(0), nj)

                    rcp = fpool.tile([128, 512], F32, tag="rcp",
                                     name=f"rcp_{g}_{i}")
                    nc.vector.reciprocal(rcp[0:64, :], vps[0][64:128, :])
                    nc.vector.reciprocal(rcp[64:128, :], vps[1][0:64, :])
                    nc.vector.tensor_mul(catt[0:64, sq0:sq0 + 512],
                                         vps[0][0:64, :], rcp[0:64, :])
                    nc.vector.tensor_mul(catt[64:128, sq0:sq0 + 512],
                                         vps[1][64:128, :], rcp[64:128, :])
                    if last:
                        tail = i == NI - 1
                        for st in range(4 * i, 4 * i + 4):
                            obs[st] = fpool.tile([128, D], BF16, tag="ob",
                                                 name=f"ob_{st}")
                            for half in range(2):
                                oneshot.append(
                                    outproj_half(st, half, tail=tail))
                    pull_gen(4)

            # All projection groups flow through one chained generator.
            # attention(g) pumps it just far enough that i-block k's inputs
            # (st-groups <= k of group g) are emitted, then keeps pulling it
            # as per-j filler — so PE never waits for late x^T quarters and
            # group g+1's projections interleave into group g's attention.
            YPG = 14  # proj_gen yields per st-group
            progress = [0] * NG
            ctxs = {}

            def chain():
                for gg in range(NG):
                    ctxs[gg] = prepare(gg)
                    for item in proj_gen(ctxs[gg]):
                        progress[gg] += 1
                        yield item

            gen_box[0] = chain()

            def pump(g, i):
                target = (i + 1) * YPG
                while gen_box[0] is not None and progress[g] < target:
                    pull_gen(1)

            for g in range(NG):
                while g not in ctxs and gen_box[0] is not None:
                    pull_gen(1)
                attention(ctxs[g], pump=lambda i, g=g: pump(g, i))
            drain_all()
    nc.finalize()
    return nc


_NC_CACHE = {}


def _get_nc():
    if "nc" not in _NC_CACHE:
        _NC_CACHE["nc"] = build_nc()
    return _NC_CACHE["nc"]


def kernel(x, Wq, bq, Wk, bk, Wv, bv, Wo, bo):
    x_bf = np.ascontiguousarray(np.asarray(x, dtype=np.float32)).astype(
        ml_dtypes.bfloat16)
    tri = np.triu(np.ones((128, 128), dtype=np.float32)).astype(
        ml_dtypes.bfloat16)
    in_maps = []
    for c in range(NCORES):
        b, half = c // 2, c % 2
        hs = [half * 8 + k for k in range(8)]
        m = {"x": x_bf[b], "tri": tri}
        # wo: per group g, rows for heads (2g, 2g+1) of this half
        wo_g = np.stack([
            np.concatenate([Wo[hs[2 * g] * DK:(hs[2 * g] + 1) * DK],
                            Wo[hs[2 * g + 1] * DK:(hs[2 * g + 1] + 1) * DK]],
                           axis=0)
            for g in range(NG)])
        m["wo"] = np.ascontiguousarray(wo_g.astype(ml_dtypes.bfloat16))
        for nm, W, bb in (("q", Wq, bq), ("k", Wk, bk), ("v", Wv, bv)):
            Wc = np.concatenate([W[h] for h in hs], axis=1)  # [D, 512]
            m["w" + nm] = np.ascontiguousarray(
                Wc.reshape(DC, 128, KC).astype(ml_dtypes.bfloat16))
            bc = np.concatenate([bb[h] for h in hs])  # [512]
            m["b" + nm] = np.ascontiguousarray(
                bc.reshape(NG, 128).T.astype(np.float32))
        in_maps.append(m)
    nc = _get_nc()
    res = run_bass_kernel_spmd(nc, in_maps, list(range(NCORES)))
    outp = np.zeros((B, S, D), dtype=np.float32)
    for c in range(NCORES):
        outp[c // 2] += np.asarray(res.results[c]["out"], dtype=np.float32)
    return outp + np.asarray(bo, dtype=np.float32)[None, None, :]


# revision 49
# speedup vs baseline: 1.0459x; 1.0020x over previous
"""Masked multi-head SDP attention, 8 NeuronCores = (batch, head-half).

B=4, S=2048, D=1024, H=16, DK=64. Core c owns batch c//2 and heads
[(c%2)*8, (c%2)*8+8), processed as 4 head-pair groups. All matmuls bf16
(full PE rate at any free width; rel-err gate 2e-2 >> bf16 noise).

- x is cast to bf16 on host and loaded straight into [d, s] layout with
  the XBAR DMA-transpose, once per core, reused by all 4 groups (no PE
  transposes or psum->sbuf copies for x^T).
- Scores for both heads of a t-tile go into one [128, 1024] PSUM pair and
  through a single 2-region exp on the Act engine; causal masking is a
  Pool-engine multiply with a duplicated triu tile (Act is the binding
  engine in attention, PE overall).
- V is stored per t-tile as [v0 | ones | v1] so each head's attn@V lhsT
  is a contiguous 128-col window whose PSUM rows hold both v@attn and the
  softmax denominator replicated 64x; normalization is reciprocal + mul
  with cross-partition-base APs (no PE broadcast, no DMA hop).
- Emission is software-pipelined: all projection groups flow through one
  chained generator that attention pumps per i-block (so PE never waits
  for late x^T quarters) and pulls as per-j filler; output-projection
  halves are pushed as fillers into the last group's stream. attn@V lags
  scores by 3-4 t-tiles so it never stalls on exp.
- The output projection accumulates all 4 groups in PSUM; each core
  writes one [S, D] bf16 partial and the host sums the two halves per
  batch in f32.
"""

import sys

sys.path.insert(0, "/opt/trn_rl_repo")

import collections
import numpy as np
import ml_dtypes

import concourse.bass as bass
import concourse.mybir as mybir
from concourse import bacc
from concourse.masks import make_identity
from concourse.tile import TileContext
from concourse.bass_utils import run_bass_kernel_spmd

B, S, D, H = 4, 2048, 1024, 16
DK = D // H  # 64
NCORES = 8
NG = 4  # head-pair groups per core
KH = 2 * DK  # 128 per group
KC = NG * KH  # 512 projection outputs per core
NT = S // 128
NI = S // 512
DC = D // 128

F32 = mybir.dt.float32
BF16 = mybir.dt.bfloat16

VW = 3 * DK  # 192: [v0 | ones | v1] per t-tile in vna


def build_nc():
    nc = bacc.Bacc("TRN2", target_bir_lowering=False, debug=False,
                   num_devices=NCORES)
    x = nc.dram_tensor("x", [S, D], BF16, kind="ExternalInput").ap()
    wq = nc.dram_tensor("wq", [DC, 128, KC], BF16, kind="ExternalInput").ap()
    wk = nc.dram_tensor("wk", [DC, 128, KC], BF16, kind="ExternalInput").ap()
    wv = nc.dram_tensor("wv", [DC, 128, KC], BF16, kind="ExternalInput").ap()
    bq = nc.dram_tensor("bq", [128, NG], F32, kind="ExternalInput").ap()
    bk = nc.dram_tensor("bk", [128, NG], F32, kind="ExternalInput").ap()
    bv = nc.dram_tensor("bv", [128, NG], F32, kind="ExternalInput").ap()
    wo = nc.dram_tensor("wo", [NG, KH, D], BF16, kind="ExternalInput").ap()
    tri = nc.dram_tensor("tri", [128, 128], BF16, kind="ExternalInput").ap()
    out = nc.dram_tensor("out", [S, D], BF16, kind="ExternalOutput").ap()

    with TileContext(nc) as tc:
        with (
            tc.tile_pool(name="const", bufs=1) as cpool,
            tc.tile_pool(name="seq", bufs=2) as qpool,
            tc.tile_pool(name="vn", bufs=2) as vpool,
            tc.tile_pool(name="attn", bufs=2) as apool,
            tc.tile_pool(name="fin", bufs=4) as fpool,
            tc.tile_pool(name="cat", bufs=1) as catpool,
            tc.tile_pool(name="pacc", bufs=2, space="PSUM") as ps_acc,
            tc.tile_pool(name="psc", bufs=2, space="PSUM") as ps_sc,
            tc.tile_pool(name="pv", bufs=1, space="PSUM") as ps_v,
        ):
            # x^T quarter 0 first, then weights, then remaining quarters —
            # minimizes time until the first projection group can run
            # (HWDGE issue is serialized at ~625ns/DMA).
            xts = [cpool.tile([128, S], BF16, tag=f"xt{dc}",
                              name=f"xt_{dc}") for dc in range(DC)]

            def xt_quarter(sh):
                for dc in range(DC):
                    nc.sync.dma_start(
                        out=xts[dc][:, sh * 512:(sh + 1) * 512],
                        in_=x[sh * 512:(sh + 1) * 512,
                              dc * 128:(dc + 1) * 128],
                        transpose=True)

            # x rows 0:512 in natural layout first (cheap contiguous DMA);
            # the otherwise-idle PE transposes them into x^T quarter 0 while
            # the weights stream in
            xn0 = cpool.tile([128, 4 * D], BF16, tag="xn0")
            nc.sync.dma_start(
                out=xn0[:].rearrange("p (s d) -> p s d", s=4),
                in_=x[0:512, :].rearrange("(s p) d -> p s d", p=128))
            w_sb = {}
            for nm, src in (("q", wq), ("k", wk), ("v", wv)):
                t = cpool.tile([128, DC * KC], BF16, tag="w" + nm,
                               name=f"w_{nm}")
                nc.sync.dma_start(
                    out=t[:].rearrange("p (c k) -> p c k", c=DC),
                    in_=src.rearrange("c p k -> p c k"))
                w_sb[nm] = t
            b_sb = {}
            for nm, src in (("q", bq), ("k", bk), ("v", bv)):
                t = cpool.tile([128, NG], F32, tag="b" + nm, name=f"b_{nm}")
                nc.sync.dma_start(out=t[:], in_=src)
                b_sb[nm] = t
            ident = cpool.tile([128, 128], BF16, tag="ident")
            make_identity(nc, ident[:])
            tri2_sb = cpool.tile([128, 256], BF16, tag="tri")
            nc.sync.dma_start(out=tri2_sb[:, 0:128], in_=tri)
            nc.sync.dma_start(out=tri2_sb[:, 128:256], in_=tri)
            # PE-transpose x^T quarter 0 from the natural-layout load (the
            # PE is otherwise idle while wk/wv stream in); quarters 1-3 use
            # the XBAR DMA-transpose as before.
            for dc in range(DC):
                tp0 = ps_acc.tile([128, 512], F32, tag="acc",
                                  name=f"tp0_{dc}")
                tp0b = tp0[:].bitcast(BF16)
                for ss in range(4):
                    nc.tensor.transpose(
                        tp0b[:, ss * 128:(ss + 1) * 128],
                        xn0[:, ss * D + dc * 128:ss * D + (dc + 1) * 128],
                        ident[:])
                nc.vector.tensor_copy(xts[dc][:, 0:512], tp0b[:, 0:512])
            for _sh in range(1, 4):
                xt_quarter(_sh)
            wo_sb = [cpool.tile([KH, D], BF16, tag=f"wo{g}", name=f"wo_{g}")
                     for g in range(NG)]
            for g in range(NG):
                nc.sync.dma_start(out=wo_sb[g][:], in_=wo[g])

            def prepare(g):
                """Per-group tiles (vna ones, qt, kt)."""
                vna = vpool.tile([128, NT * VW], BF16, tag="vna",
                                 name=f"vna_{g}")
                vna_r = vna[:].rearrange("p (j g c) -> p j g c", j=NT, g=3)
                nc.gpsimd.memset(vna_r[:, :, 1:2, :], 1.0)
                qt = qpool.tile([128, S], BF16, tag="qt", name=f"qt_{g}")
                kt = qpool.tile([128, S], BF16, tag="kt", name=f"kt_{g}")
                return dict(g=g, vna=vna, vna_r=vna_r, qt=qt, kt=kt)

            def proj_gen(ctx):
                g = ctx["g"]
                for st in range(NI):
                    sl = slice(st * 512, (st + 1) * 512)
                    for nm in ("q", "k", "v"):
                        acc = ps_acc.tile([128, 512], F32, tag="acc",
                                          name=f"acc_{g}_{st}_{nm}")
                        for dc0 in range(0, DC, 2):
                            for dc in (dc0, dc0 + 1):
                                nc.tensor.matmul(
                                    acc[:],
                                    w_sb[nm][:, dc * KC + g * KH:
                                             dc * KC + (g + 1) * KH],
                                    xts[dc][:, sl], start=(dc == 0),
                                    stop=(dc == DC - 1))
                            yield
                        if nm != "v":
                            dst = ctx["qt"] if nm == "q" else ctx["kt"]
                            nc.vector.tensor_scalar_add(
                                dst[:, sl], acc[:], b_sb[nm][:, g:g + 1])
                        else:
                            vtt = fpool.tile([128, 512], BF16, tag="vtt",
                                             name=f"vtt_{g}_{st}")
                            nc.vector.tensor_scalar_add(
                                vtt[:], acc[:], b_sb["v"][:, g:g + 1])
                            tp = ps_acc.tile([128, 512], F32, tag="acc",
                                             name=f"tp_{g}_{st}")
                            tpb = tp[:].bitcast(BF16)
                            for q in range(4):
                                nc.tensor.transpose(
                                    tpb[:, q * 128:(q + 1) * 128],
                                    vtt[:, q * 128:(q + 1) * 128], ident[:])
                            yield
                            tp_r = tpb[:, 0:512].rearrange(
                                "p (q g c) -> p q g c", q=4, g=2)
                            js = slice(st * 4, (st + 1) * 4)
                            nc.vector.tensor_copy(
                                ctx["vna_r"][:, js, 0:1, :],
                                tp_r[:, :, 0:1, :])
                            nc.vector.tensor_copy(
                                ctx["vna_r"][:, js, 2:3, :],
                                tp_r[:, :, 1:2, :])
                            yield

            oneshot = collections.deque()
            gen_box = [None]

            def pull_gen(n=1):
                for _ in range(n):
                    if gen_box[0] is not None:
                        try:
                            next(gen_box[0])
                            continue
                        except StopIteration:
                            gen_box[0] = None
                    if oneshot:
                        oneshot.popleft()()

            def drain_all():
                while oneshot or gen_box[0] is not None:
                    pull_gen(1)

            def flush_v(vps, vna, item, nj):
                j, off, at = item
                for h in range(2):
                    base = j * VW + h * DK
                    nc.tensor.matmul(
                        vps[h][:, off:512], vna[:, base:base + 2 * DK],
                        at[:, h * 512 + off:h * 512 + 512],
                        start=(j == 0), stop=(j == nj - 1))

            catts = [None] * NG

            def outproj_half(st, half, tail=False):
                """pw[128,512] = sum_g catt_g[:, st] @ wo_g[:, half]."""
                def emit():
                    ob = obs[st]
                    pw = ps_acc.tile([128, 512], F32, tag="acc",
                                     name=f"pw_{st}_{half}")
                    for g in range(NG):
                        nc.tensor.matmul(
                            pw[:], catts[g][:, st * 128:(st + 1) * 128],
                            wo_sb[g][:, half * 512:(half + 1) * 512],
                            start=(g == 0), stop=(g == NG - 1))
                    dst = ob[:, half * 512:(half + 1) * 512]
                    if tail and half == 0:
                        nc.scalar.copy(dst, pw[:])
                    else:
                        nc.vector.tensor_copy(dst, pw[:])
                    if tail and st >= 14:
                        # split the last tiles' writes so the final DMA
                        # starts as soon as each half lands
                        nc.sync.dma_start(
                            out=out[st * 128:(st + 1) * 128,
                                    half * 512:(half + 1) * 512], in_=dst)
                    elif half == 1:
                        nc.sync.dma_start(
                            out=out[st * 128:(st + 1) * 128, :], in_=ob[:])
                return emit

            obs = {}

            def attention(ctx, pump=None):
                g = ctx["g"]
                qt, kt, vna = ctx["qt"], ctx["kt"], ctx["vna"]
                catt = catpool.tile([128, S], BF16, tag=f"catt{g}",
                                    name=f"catt_{g}")
                catts[g] = catt
                last = g == NG - 1
                for i in range(NI):
                    if pump is not None:
                        pump(i)
                    nj = 4 * i + 4
                    vps = [ps_v.tile([128, 512], F32, tag=f"v{h}",
                                     name=f"vp_{g}_{i}_{h}")
                           for h in range(2)]
                    sq0 = i * 512
                    pend = []
                    for j in range(nj):
                        q = j - 4 * i
                        off = 128 * q if q >= 0 else 0
                        sp = ps_sc.tile([128, 1024], F32, tag="sc",
                                        name=f"sp_{g}_{i}_{j}")
                        for h in range(2):
                            ks = slice(h * DK, (h + 1) * DK)
                            nc.tensor.matmul(
                                sp[:, h * 512 + off:h * 512 + 512],
                                kt[ks, j * 128:(j + 1) * 128],
                                qt[ks, sq0 + off:sq0 + 512],
                                start=True, stop=True)
                        at = apool.tile([128, 1024], BF16, tag=f"at{j}",
                                        name=f"at_{g}_{i}_{j}")
                        sp2 = sp[:].rearrange("p (r c) -> p r c", r=2)
                        at2 = at[:].rearrange("p (r c) -> p r c", r=2)
                        nc.scalar.activation(
                            at2[:, :, off:512], sp2[:, :, off:512],
                            mybir.ActivationFunctionType.Exp, scale=0.125)
                        if q >= 0:
                            nc.vector.tensor_mul(
                                at2[:, :, off:off + 128],
                                at2[:, :, off:off + 128],
                                tri2_sb[:].rearrange("p (r c) -> p r c", r=2))
                        pend.append((j, off, at))
                        if len(pend) > 3:
                            flush_v(vps, vna, pend.pop(0), nj)
                        pull_gen(1)
                    while pend:
                        flush_v(vps, vna, pend.pop(0), nj)

                    rcp = fpool.tile([128, 512], F32, tag="rcp",
                                     name=f"rcp_{g}_{i}")
                    nc.vector.reciprocal(rcp[0:64, :], vps[0][64:128, :])
                    nc.vector.reciprocal(rcp[64:128, :], vps[1][0:64, :])
                    nc.vector.tensor_mul(catt[0:64, sq0:sq0 + 512],
                                         vps[0][0:64, :], rcp[0:64, :])
                    nc.vector.tensor_mul(catt[64:128, sq0:sq0 + 512],
                                         vps[1][64:128, :], rcp[64:128, :])
                    if last:
                        tail = i == NI - 1
                        for st in range(4 * i, 4 * i + 4):
                            obs[st] = fpool.tile([128, D], BF16, tag="ob",
                                                 name=f"ob_{st}")
                            for half in range(2):
                                oneshot.append(
                                    outproj_half(st, half, tail=tail))
                    pull_gen(4)

            # All projection groups flow through one chained generator.
            # attention(g) pumps it just far enough that i-block k's inputs
            # (st-groups <= k of group g) are emitted, then keeps pulling it
            # as per-j filler — so PE never waits for late x^T quarters and
            # group g+1's projections interleave into group g's attention.
            YPG = 14  # proj_gen yields per st-group
            progress = [0] * NG
            ctxs = {}

            def chain():
                for gg in range(NG):
                    ctxs[gg] = prepare(gg)
                    for item in proj_gen(ctxs[gg]):
                        progress[gg] += 1
                        yield item

            gen_box[0] = chain()

            def pump(g, i):
                target = (i + 1) * YPG
                while gen_box[0] is not None and progress[g] < target:
                    pull_gen(1)

            for g in range(NG):
                while g not in ctxs and gen_box[0] is not None:
                    pull_gen(1)
                attention(ctxs[g], pump=lambda i, g=g: pump(g, i))
            drain_all()
    nc.finalize()
    return nc


_NC_CACHE = {}


def _get_nc():
    if "nc" not in _NC_CACHE:
        _NC_CACHE["nc"] = build_nc()
    return _NC_CACHE["nc"]


def kernel(x, Wq, bq, Wk, bk, Wv, bv, Wo, bo):
    x_bf = np.ascontiguousarray(np.asarray(x, dtype=np.float32)).astype(
        ml_dtypes.bfloat16)
    tri = np.triu(np.ones((128, 128), dtype=np.float32)).astype(
        ml_dtypes.bfloat16)
    in_maps = []
    for c in range(NCORES):
        b, half = c // 2, c % 2
        hs = [half * 8 + k for k in range(8)]
        m = {"x": x_bf[b], "tri": tri}
        # wo: per group g, rows for heads (2g, 2g+1) of this half
        wo_g = np.stack([
            np.concatenate([Wo[hs[2 * g] * DK:(hs[2 * g] + 1) * DK],
                            Wo[hs[2 * g + 1] * DK:(hs[2 * g + 1] + 1) * DK]],
                           axis=0)
            for g in range(NG)])
        m["wo"] = np.ascontiguousarray(wo_g.astype(ml_dtypes.bfloat16))
        for nm, W, bb in (("q", Wq, bq), ("k", Wk, bk), ("v", Wv, bv)):
            Wc = np.concatenate([W[h] for h in hs], axis=1)  # [D, 512]
            m["w" + nm] = np.ascontiguousarray(
                Wc.reshape(DC, 128, KC).astype(ml_dtypes.bfloat16))
            bc = np.concatenate([bb[h] for h in hs])  # [512]
            m["b" + nm] = np.ascontiguousarray(
                bc.reshape(NG, 128).T.astype(np.float32))
        in_maps.append(m)
    nc = _get_nc()
    res = run_bass_kernel_spmd(nc, in_maps, list(range(NCORES)))
    outp = np.zeros((B, S, D), dtype=np.float32)
    for c in range(NCORES):
        outp[c // 2] += np.asarray(res.results[c]["out"], dtype=np.float32)
    return outp + np.asarray(bo, dtype=np.float32)[None, None, :]
